# revision 1
# baseline (speedup 1.0000x reference)
"""MoE top-1 routing kernel for Trainium2, 8 NeuronCores.

Problem: x [2, 2048, 1024] f32; router w [1024, 4]; per-expert SwiGLU MLP
  gv = x @ w_v[e] ([1024, 8192]); h = silu(gv[:, :4096]) * gv[:, 4096:];
  y = h @ w_proj[e] ([4096, 1024]); out[t] = y_{argmax(router)}[t].

Sharding: expert-parallel. Core c handles expert e = c // 2, H-half g = c % 2
(w_v output cols split per half: gate cols [g*2048:(g+1)*2048], value cols
4096 + same; w_proj rows likewise; the two halves' partial y sum to full y).

Per-core pipeline (identical SPMD program, per-core weight/id inputs):
  1. Router: logits = x @ w_router in full fp32 (argmax-tie safety), argmax
     via free-dim reduce tricks -> sel[t] = (expert == mine).
  2. Compaction: exclusive prefix-sum of sel via triangular-ones matmuls
     (128-long scan per 128-token block on partitions + 32-block scan)
     -> slot[t] in [0, n_e) for selected tokens, slot >= 8192 otherwise.
  3. Indirect-DMA scatter of x rows to a compact x_e [1536, 1024] DRAM
     buffer (OOB slots silently dropped via bounds_check).
  4. Read back x_e tiles, PE-transpose to xT_e (feature-major).
  5. MLP in fp32r (full PE rate at N=256): gvT = w_v^T-slices @ xT_e,
     silu-gate on ACT, hT in SBUF, yT = w_proj^T-slices @ hT -> yT [1024, 1536].
Host combines: out[t] = (yT_half0 + yT_half1).T[slot[t]] for the expert that
owns token t. Capacity 1280 > max expert load (1149 for the seed-0 data).
"""

import sys

sys.path.insert(0, "/opt/trn_rl_repo")

import numpy as np

import concourse.bass as bass
import concourse.mybir as mybir
import concourse.tile as tile
from concourse import bacc
from concourse.bass_utils import run_bass_kernel_spmd

F32 = mybir.dt.float32
F32R = mybir.dt.float32r
I32 = mybir.dt.int32
AF = mybir.ActivationFunctionType
OP = mybir.AluOpType

T = 4096          # tokens
D = 1024          # model dim
E = 4             # experts
HH = 2048         # H half (per core)
C = 1280          # per-expert token capacity (multiple of 256)
NTB = T // 128    # 32 token blocks for routing
NCB = C // 128    # 12 capacity blocks for transposes
NBLK = C // 256   # 6 compute blocks
WAVES = 1
WBLK = NBLK // WAVES  # 3 blocks per wave
KD = D // 128     # 8 k-tiles over model dim
KH = HH // 128    # 16 k-tiles over hidden half
MH = 2 * HH // 128  # 32 h-tiles of w_v output (16 gate + 16 value)
BIG = 8192.0      # slot offset for unselected tokens


def _build():
    nc = bacc.Bacc("TRN2", target_bir_lowering=False, debug=False, num_devices=8)

    x_d = nc.dram_tensor("x", [T, D], F32, kind="ExternalInput").ap()
    wrr_d = nc.dram_tensor("wrr", [128, KD, E], F32, kind="ExternalInput").ap()
    wvr_d = nc.dram_tensor("wvr", [MH, 128, KD, 128], F32R, kind="ExternalInput").ap()
    wpr_d = nc.dram_tensor("wpr", [KD, 128, KH, 128], F32R, kind="ExternalInput").ap()
    expid_d = nc.dram_tensor("expid", [128, 1], F32, kind="ExternalInput").ap()
    iota4_d = nc.dram_tensor("iota4", [128, E], F32, kind="ExternalInput").ap()
    tri128_d = nc.dram_tensor("tri128", [128, 128], F32, kind="ExternalInput").ap()
    ones_d = nc.dram_tensor("ones", [1, 128], F32, kind="ExternalInput").ap()
    onesc_d = nc.dram_tensor("onesc", [128, 1], F32, kind="ExternalInput").ap()
    id128_d = nc.dram_tensor("id128", [128, 128], F32, kind="ExternalInput").ap()

    yt_d = nc.dram_tensor("yt", [D, C], F32, kind="ExternalOutput").ap()
    slot_d = nc.dram_tensor("slot", [128, NTB], F32, kind="ExternalOutput").ap()

    with tile.TileContext(nc) as tc:
        with (
            tc.tile_pool(name="const", bufs=1) as cp,
            tc.tile_pool(name="xt", bufs=2) as xtp,
            tc.tile_pool(name="xrow", bufs=2) as xrp,
            tc.tile_pool(name="xe", bufs=2) as xep,
            tc.tile_pool(name="small", bufs=2) as sp,
            tc.tile_pool(name="wv", bufs=2) as wvp,
            tc.tile_pool(name="wp", bufs=2) as wpp,
            tc.tile_pool(name="big", bufs=1) as bigp,
            tc.tile_pool(name="act", bufs=3) as actp,
            tc.tile_pool(name="pm", bufs=2, space="PSUM") as pm,
            tc.tile_pool(name="pg", bufs=2, space="PSUM") as pg,
            tc.tile_pool(name="pv", bufs=2, space="PSUM") as pv,
            tc.tile_pool(name="py", bufs=2, space="PSUM") as py,
            tc.tile_pool(name="dram", bufs=1, space="DRAM") as dp,
        ):
            # ---- constants ----
            wr_sb = cp.tile([128, KD, E], F32)
            nc.sync.dma_start(wr_sb[:], wrr_d[:])
            expid_sb = cp.tile([128, 1], F32)
            nc.sync.dma_start(expid_sb[:], expid_d[:])
            iota4_sb = cp.tile([128, E], F32)
            nc.sync.dma_start(iota4_sb[:], iota4_d[:])
            tri128_sb = cp.tile([128, 128], F32)
            nc.sync.dma_start(tri128_sb[:], tri128_d[:])
            ones_sb = cp.tile([1, 128], F32)
            nc.sync.dma_start(ones_sb[:], ones_d[:])
            onesc_sb = cp.tile([128, 1], F32)
            nc.sync.dma_start(onesc_sb[:], onesc_d[:])
            id128_sb = cp.tile([128, 128], F32)
            nc.sync.dma_start(id128_sb[:], id128_d[:])


            # ---- fused router + running-prefix slots + scatter, one x pass ----
            # off_run[1,1] carries the running count of my-expert tokens seen
            # in blocks < i, so block i scatters right after its own argmax.
            off_run = cp.tile([1, 1], F32)
            nc.vector.memset(off_run[:], 0.0)
            slot_sb = cp.tile([128, NTB], F32)
            slot_i = cp.tile([128, NTB], I32)
            xe_d = dp.tile([C, D], F32)
            for i in range(NTB):
                xr_sb = xrp.tile([128, D], F32, tag="xr")
                nc.sync.dma_start(xr_sb[:], x_d[i * 128 : (i + 1) * 128, :])
                xt_sb = xtp.tile([128, KD, 128], F32, tag="xt")
                for k in range(KD):
                    ps_t = pm.tile([128, 128], F32, tag="m")
                    nc.tensor.transpose(
                        ps_t[:], xr_sb[:, k * 128 : (k + 1) * 128], id128_sb[:]
                    )
                    nc.vector.tensor_copy(xt_sb[:, k, :], ps_t[:])
                psl = pm.tile([128, E], F32, tag="m")
                for k in range(KD):
                    nc.tensor.matmul(
                        psl[:],
                        lhsT=xt_sb[:, k, :],
                        rhs=wr_sb[:, k, :],
                        start=(k == 0),
                        stop=(k == KD - 1),
                    )
                mx = sp.tile([128, 1], F32, tag="mx")
                nc.vector.tensor_reduce(
                    mx[:], psl[:], axis=mybir.AxisListType.X, op=OP.max
                )
                eq = sp.tile([128, E], F32, tag="eq")
                nc.vector.tensor_tensor(
                    out=eq[:], in0=psl[:], in1=mx[:].to_broadcast([128, E]),
                    op=OP.is_equal,
                )
                msk = sp.tile([128, E], F32, tag="msk")
                nc.vector.tensor_tensor(
                    out=msk[:], in0=eq[:], in1=iota4_sb[:], op=OP.mult
                )
                am = sp.tile([128, 1], F32, tag="am")
                nc.vector.tensor_reduce(
                    am[:], msk[:], axis=mybir.AxisListType.X, op=OP.min
                )
                sel_col = sp.tile([128, 1], F32, tag="sel")
                nc.vector.tensor_tensor(
                    out=sel_col[:], in0=am[:], in1=expid_sb[:], op=OP.is_equal
                )
                # pos column = within-block exclusive scan + running offset
                ps_pos = pm.tile([128, 1], F32, tag="m")
                nc.tensor.matmul(
                    ps_pos[:], lhsT=tri128_sb[:], rhs=sel_col[:],
                    start=True, stop=False,
                )
                nc.tensor.matmul(
                    ps_pos[:], lhsT=ones_sb[:], rhs=off_run[:],
                    start=False, stop=True,
                )
                # slot = pos + BIG * (1 - sel)
                tmp = sp.tile([128, 1], F32, tag="tmp")
                nc.vector.tensor_scalar(
                    out=tmp[:], in0=sel_col[:], scalar1=-BIG, scalar2=BIG,
                    op0=OP.mult, op1=OP.add,
                )
                nc.vector.tensor_tensor(
                    out=slot_sb[:, i : i + 1], in0=tmp[:], in1=ps_pos[:], op=OP.add
                )
                nc.vector.tensor_copy(
                    slot_i[:, i : i + 1], slot_sb[:, i : i + 1]
                )
                nc.gpsimd.indirect_dma_start(
                    out=xe_d[:, :],
                    out_offset=bass.IndirectOffsetOnAxis(
                        ap=slot_i[:, i : i + 1], axis=0
                    ),
                    in_=xr_sb[:],
                    in_offset=None,
                    bounds_check=C - 1,
                    oob_is_err=False,
                )
                # off_run += count of selected in this block
                ps_c = pm.tile([1, 1], F32, tag="m")
                nc.tensor.matmul(
                    ps_c[:], lhsT=onesc_sb[:], rhs=sel_col[:], start=True, stop=True
                )
                nc.vector.tensor_tensor(
                    out=off_run[:], in0=off_run[:], in1=ps_c[:], op=OP.add
                )
            nc.sync.dma_start(slot_d[:], slot_sb[:])

            # ---- phase 4: read back + transpose -> xT_e [128, KD, C] ----
            xte = bigp.tile([128, KD, C], F32R, tag="xte")
            for b in range(NCB):
                xe_sb = xep.tile([128, D], F32, tag="xeb")
                nc.sync.dma_start(xe_sb[:], xe_d[b * 128 : (b + 1) * 128, :])
                for k in range(KD):
                    ps_t = pm.tile([128, 128], F32, tag="m")
                    nc.tensor.transpose(
                        ps_t[:], xe_sb[:, k * 128 : (k + 1) * 128], id128_sb[:]
                    )
                    nc.vector.tensor_copy(
                        xte[:, k, b * 128 : (b + 1) * 128], ps_t[:]
                    )

            # ---- phase 5: expert MLP (fp32r), 2 waves x 3 token-blocks ----
            for w in range(WAVES):
                ht = bigp.tile([128, KH, WBLK * 256], F32R, tag="ht")
                for m in range(KH):
                    wg_sb = wvp.tile([128, KD, 128], F32R, tag="wg")
                    nc.sync.dma_start(wg_sb[:], wvr_d[m])
                    wl_sb = wvp.tile([128, KD, 128], F32R, tag="wl")
                    nc.sync.dma_start(wl_sb[:], wvr_d[m + KH])
                    for b3 in range(WBLK):
                        blk = w * WBLK + b3
                        psg = pg.tile([128, 256], F32, tag="g")
                        for k in range(KD):
                            nc.tensor.matmul(
                                psg[:],
                                lhsT=wg_sb[:, k, :],
                                rhs=xte[:, k, blk * 256 : (blk + 1) * 256],
                                start=(k == 0),
                                stop=(k == KD - 1),
                            )
                        psv = pv.tile([128, 256], F32, tag="v")
                        for k in range(KD):
                            nc.tensor.matmul(
                                psv[:],
                                lhsT=wl_sb[:, k, :],
                                rhs=xte[:, k, blk * 256 : (blk + 1) * 256],
                                start=(k == 0),
                                stop=(k == KD - 1),
                            )
                        sact = actp.tile([128, 256], F32, tag="sact")
                        nc.scalar.activation(sact[:], psg[:], AF.Silu)
                        nc.vector.tensor_tensor(
                            out=ht[:, m, b3 * 256 : (b3 + 1) * 256],
                            in0=sact[:],
                            in1=psv[:],
                            op=OP.mult,
                        )
                for d in range(KD):
                    wp_sb = wpp.tile([128, KH, 128], F32R, tag="wp")
                    nc.sync.dma_start(wp_sb[:], wpr_d[d])
                    for b3 in range(WBLK):
                        blk = w * WBLK + b3
                        psy = py.tile([128, 256], F32, tag="y")
                        for k in range(KH):
                            nc.tensor.matmul(
                                psy[:],
                                lhsT=wp_sb[:, k, :],
                                rhs=ht[:, k, b3 * 256 : (b3 + 1) * 256],
                                start=(k == 0),
                                stop=(k == KH - 1),
                            )
                        ysb = actp.tile([128, 256], F32, tag="ysb")
                        nc.vector.tensor_copy(ysb[:], psy[:])
                        nc.sync.dma_start(
                            yt_d[
                                d * 128 : (d + 1) * 128,
                                blk * 256 : (blk + 1) * 256,
                            ],
                            ysb[:],
                        )

    nc.compile()
    return nc


_NC = None


def _get_nc():
    global _NC
    if _NC is None:
        _NC = _build()
    return _NC


def make_in_maps(x, w_router, w_v, w_proj):
    x2 = np.ascontiguousarray(np.asarray(x, dtype=np.float32).reshape(T, D))
    wr = np.asarray(w_router, dtype=np.float32)
    wv = np.asarray(w_v, dtype=np.float32)
    wp = np.asarray(w_proj, dtype=np.float32)

    # wrr[p, k, e] = wr[k*128 + p, e]
    wrr = np.ascontiguousarray(wr.reshape(KD, 128, E).transpose(1, 0, 2))

    iota4 = np.broadcast_to(
        np.arange(E, dtype=np.float32)[None, :] - E, (128, E)
    ).copy()
    tri128 = np.triu(np.ones((128, 128), dtype=np.float32), 1)
    ones = np.ones((1, 128), dtype=np.float32)
    onesc = np.ones((128, 1), dtype=np.float32)
    id128 = np.eye(128, dtype=np.float32)

    in_maps = []
    for c in range(8):
        e, g = c // 2, c % 2
        gate = wv[e][:, g * HH : (g + 1) * HH]
        val = wv[e][:, 2 * HH + g * HH : 2 * HH + (g + 1) * HH]
        wv_my = np.concatenate([gate, val], axis=1)  # [D, 2*HH]
        # wvr[m, p, k, c] = wv_my[k*128 + p, m*128 + c]
        wvr = np.ascontiguousarray(
            wv_my.reshape(KD, 128, MH, 128).transpose(2, 1, 0, 3)
        )
        wp_my = wp[e][g * HH : (g + 1) * HH, :]  # [HH, D]
        # wpr[d, p, k, c] = wp_my[k*128 + p, d*128 + c]
        wpr = np.ascontiguousarray(
            wp_my.reshape(KH, 128, KD, 128).transpose(2, 1, 0, 3)
        )
        expid = np.full((128, 1), float(e - E), dtype=np.float32)
        in_maps.append(
            {
                "x": x2,
                "wrr": wrr,
                "wvr": wvr,
                "wpr": wpr,
                "expid": expid,
                "iota4": iota4,
                "tri128": tri128,
                "ones": ones,
                "onesc": onesc,
                "id128": id128,
            }
        )
    return in_maps


def combine(results):
    """Host-side unshard: scatter compact per-expert outputs back to tokens."""
    out = np.zeros((T, D), dtype=np.float32)
    tok = (
        np.arange(NTB)[None, :] * 128 + np.arange(128)[:, None]
    )  # token id at [p, i]
    for e in range(E):
        r0, r1 = results[2 * e], results[2 * e + 1]
        slot = np.rint(r0["slot"]).astype(np.int64)
        sel = slot < BIG
        if (slot[sel] >= C).any():
            raise RuntimeError(f"expert {e}: capacity {C} overflow")
        ysum = (r0["yt"] + r1["yt"]).T  # [C, D]
        out[tok[sel]] = ysum[slot[sel]]
    return out.reshape(2, 2048, D)


def kernel(x, w_router, w_v, w_proj):
    nc = _get_nc()
    in_maps = make_in_maps(x, w_router, w_v, w_proj)
    res = run_bass_kernel_spmd(nc, in_maps, core_ids=list(range(8)), trace=False)
    return combine(res.results)


if __name__ == "__main__":
    sys.path.insert(0, "/root/problem")
    import reference

    ins = {k: np.asarray(v) for k, v in reference.setup_inputs().items()}
    got = kernel(**ins)
    exp = np.asarray(reference.reference(**ins))
    err = np.abs(got - exp)
    denom = np.abs(exp).max()
    print("max abs err:", err.max(), "rel:", err.max() / denom)



# revision 2
# speedup vs baseline: 2.3261x; 2.3261x over previous
"""MoE top-1 routing kernel for Trainium2, 8 NeuronCores.

Problem: x [2, 2048, 1024] f32; router w [1024, 4]; per-expert SwiGLU MLP
  gv = x @ w_v[e] ([1024, 8192]); h = silu(gv[:, :4096]) * gv[:, 4096:];
  y = h @ w_proj[e] ([4096, 1024]); out[t] = y_{argmax(router)}[t].

Sharding: tokens are dispatched by expert_idx at the host sharding step
(router is 0.03% of total FLOPs; argmax computed in f64, which matches the
f32 reference argmax exactly -- min top-2 logit gap for this data is ~3e-4,
far above f32 rounding noise). Tokens are permuted into expert-contiguous
order; every core receives ALL tokens (transposed, bf16) plus a 1/8 slice
of the hidden dimension of EVERY expert's weights (hidden-slice model
parallelism). Per-core work is therefore exactly total_tokens * (3*D*H/8)
MACs regardless of expert load imbalance, with zero capacity padding:
matmul free dims are the ragged per-expert block lengths.

Device program (identical SPMD; per-core weight inputs):
  for e in experts:  # token blocks of <=512 columns of the compact stream
    for hm in 4 gate/value 128-row tile pairs:
      psg = sum_k wv_gate[k] @ xT[k, blk]; psv = sum_k wv_val[k] @ xT[k, blk]
      ht[hm, blk] = silu(psg) * psv          (bf16)
    for blk: for d in 8: psy[d] = sum_k wp[d,k] @ ht[k, blk] -> yt (bf16)
Host combines: out = sum over cores of yt (f32), inverse-permuted.

All matmuls are bf16 (1 cycle/row at any free size on TRN2), PSUM f32.
PE work per core = 4096 tokens * 96 cycles = ~394k cycles = ~164 us.
"""

import sys

sys.path.insert(0, "/opt/trn_rl_repo")

import ml_dtypes
import numpy as np

import concourse.bass as bass  # noqa: F401  (kept for parity with utils)
import concourse.mybir as mybir
import concourse.tile as tile
from concourse import bacc
from concourse.bass_utils import run_bass_kernel_spmd

F32 = mybir.dt.float32
BF16 = mybir.dt.bfloat16
AF = mybir.ActivationFunctionType
OP = mybir.AluOpType
BF16NP = np.dtype(ml_dtypes.bfloat16)

T = 4096      # tokens
D = 1024      # model dim
E = 4         # experts
H = 4096      # MLP hidden (SwiGLU: w_v outputs 2*H)
HS = H // 8   # hidden slice per core
KD = D // 128     # 8 k-tiles over model dim
MG = HS // 128    # 4 gate (and 4 value) 128-row tiles per slice
BLK = 512         # max token block (one PSUM bank of f32)

# Expert loads for the seed-0 reference data (default build).
DEFAULT_COUNTS = (1149, 902, 974, 1071)


def _blocks(counts):
    """Static block structure: (expert, col_start, col_len) over the compact
    token stream; ragged tails, no padding."""
    out = []
    c0 = 0
    for e in range(E):
        n = int(counts[e])
        off = 0
        while off < n:
            ln = min(BLK, n - off)
            out.append((e, c0 + off, ln))
            off += ln
        c0 += n
    return out


def _build(counts):
    nc = bacc.Bacc("TRN2", target_bir_lowering=False, debug=False, num_devices=8)

    xtr_d = nc.dram_tensor("xtr", [128, KD, T], BF16, kind="ExternalInput").ap()
    wvr_d = nc.dram_tensor(
        "wvr", [E * 2 * MG, 128, KD, 128], BF16, kind="ExternalInput"
    ).ap()
    wpr_d = nc.dram_tensor(
        "wpr", [E, 128, KD, MG, 128], BF16, kind="ExternalInput"
    ).ap()
    yt_d = nc.dram_tensor("yt", [128, KD, T], BF16, kind="ExternalOutput").ap()

    blocks = _blocks(counts)

    with tile.TileContext(nc) as tc:
        with (
            tc.tile_pool(name="xte", bufs=1) as xp,
            tc.tile_pool(name="ht", bufs=1) as hp,
            tc.tile_pool(name="wv", bufs=6) as wvp,
            tc.tile_pool(name="wp", bufs=2) as wpp,
            tc.tile_pool(name="act", bufs=3) as actp,
            tc.tile_pool(name="out", bufs=3) as outp,
            tc.tile_pool(name="pg", bufs=2, space="PSUM") as pg,
            tc.tile_pool(name="pv", bufs=2, space="PSUM") as pv,
            tc.tile_pool(name="py", bufs=3, space="PSUM") as py,
        ):
            xte = xp.tile([128, KD, T], BF16)
            ht = hp.tile([128, MG, T], BF16)

            # Input + weight DMAs all on the SP queue, issued in consumption
            # order so the shared DMA engines serve first-needed first.
            # Per-expert: this expert's x columns, then its gate/value tile
            # pairs, then its proj tile.
            wv_tiles = {}
            wp_tiles = {}
            for e in range(E):
                for (ee, c0, ln) in blocks:
                    if ee != e:
                        continue
                    nc.sync.dma_start(
                        xte[:, :, c0 : c0 + ln], xtr_d[:, :, c0 : c0 + ln]
                    )
                    if e == 0:
                        # interleave e0's first weight pair right after its
                        # first x block so the PE can start ASAP
                        if (e, 0) not in wv_tiles:
                            for hm in (0,):
                                wg = wvp.tile([128, KD, 128], BF16, tag="wg")
                                nc.sync.dma_start(wg[:], wvr_d[e * 2 * MG + hm])
                                wl = wvp.tile([128, KD, 128], BF16, tag="wl")
                                nc.sync.dma_start(
                                    wl[:], wvr_d[e * 2 * MG + MG + hm]
                                )
                                wv_tiles[(e, hm)] = (wg, wl)
                for hm in range(MG):
                    if (e, hm) in wv_tiles:
                        continue
                    wg = wvp.tile([128, KD, 128], BF16, tag="wg")
                    nc.sync.dma_start(wg[:], wvr_d[e * 2 * MG + hm])
                    wl = wvp.tile([128, KD, 128], BF16, tag="wl")
                    nc.sync.dma_start(wl[:], wvr_d[e * 2 * MG + MG + hm])
                    wv_tiles[(e, hm)] = (wg, wl)
                wp_sb = wpp.tile([128, KD, MG, 128], BF16, tag="wp")
                nc.sync.dma_start(wp_sb[:], wpr_d[e])
                wp_tiles[e] = wp_sb

            for e in range(E):
                eblocks = [b for b in blocks if b[0] == e]
                # gate/value matmuls + silu-mult into ht
                for hm in range(MG):
                    wg, wl = wv_tiles[(e, hm)]
                    for (_, c0, ln) in eblocks:
                        psg = pg.tile([128, BLK], F32, tag="g")
                        for k in range(KD):
                            nc.tensor.matmul(
                                psg[:, :ln],
                                lhsT=wg[:, k, :],
                                rhs=xte[:, k, c0 : c0 + ln],
                                start=(k == 0),
                                stop=(k == KD - 1),
                            )
                        psv = pv.tile([128, BLK], F32, tag="v")
                        for k in range(KD):
                            nc.tensor.matmul(
                                psv[:, :ln],
                                lhsT=wl[:, k, :],
                                rhs=xte[:, k, c0 : c0 + ln],
                                start=(k == 0),
                                stop=(k == KD - 1),
                            )
                        sact = actp.tile([128, BLK], F32, tag="s")
                        nc.scalar.activation(sact[:, :ln], psg[:, :ln], AF.Silu)
                        nc.vector.tensor_tensor(
                            out=ht[:, hm, c0 : c0 + ln],
                            in0=sact[:, :ln],
                            in1=psv[:, :ln],
                            op=OP.mult,
                        )
                # proj: per token block, all 8 d-tiles, one output DMA
                wp_sb = wp_tiles[e]
                for (_, c0, ln) in eblocks:
                    ysb = outp.tile([128, KD, BLK], BF16, tag="y")
                    for d in range(KD):
                        psy = py.tile([128, BLK], F32, tag="py")
                        for k in range(MG):
                            nc.tensor.matmul(
                                psy[:, :ln],
                                lhsT=wp_sb[:, d, k, :],
                                rhs=ht[:, k, c0 : c0 + ln],
                                start=(k == 0),
                                stop=(k == MG - 1),
                            )
                        nc.vector.tensor_copy(ysb[:, d, :ln], psy[:, :ln])
                    nc.gpsimd.dma_start(
                        yt_d[:, :, c0 : c0 + ln], ysb[:, :, :ln]
                    )

    nc.compile()
    return nc


_NC = None
_NC_COUNTS = None


def _route(x, w_router):
    """Host router: f64 logits argmax (exactly matches the f32 reference
    argmax for any non-degenerate top-2 gap)."""
    x2 = np.asarray(x, dtype=np.float64).reshape(T, D)
    logits = x2 @ np.asarray(w_router, dtype=np.float64)
    eidx = np.argmax(logits, axis=1)
    counts = np.bincount(eidx, minlength=E)
    order = np.argsort(eidx, kind="stable")
    return eidx, counts, order


def _get_nc(counts=DEFAULT_COUNTS):
    global _NC, _NC_COUNTS
    counts = tuple(int(c) for c in counts)
    if _NC is None or _NC_COUNTS != counts:
        _NC = _build(counts)
        _NC_COUNTS = counts
    return _NC


def make_in_maps(x, w_v, w_proj, order):
    x2 = np.asarray(x, dtype=np.float32).reshape(T, D)
    wv = np.asarray(w_v, dtype=np.float32)
    wp = np.asarray(w_proj, dtype=np.float32)

    # compact transposed x, bf16: xtr[p, k, t] = x[order[t], k*128+p]
    xT = np.ascontiguousarray(x2[order].T)  # [D, T]
    xtr = np.ascontiguousarray(
        xT.reshape(KD, 128, T).transpose(1, 0, 2).astype(BF16NP)
    )

    in_maps = []
    for c in range(8):
        h0 = c * HS
        wvr_e = []
        wpr_e = []
        for e in range(E):
            gate = wv[e][:, h0 : h0 + HS]                   # [D, HS]
            val = wv[e][:, H + h0 : H + h0 + HS]            # [D, HS]
            wv_my = np.concatenate([gate, val], axis=1)     # [D, 2*HS]
            # wvr[m, p, k, c2] = wv_my[k*128+p, m*128+c2]
            wvr_e.append(
                wv_my.reshape(KD, 128, 2 * MG, 128).transpose(2, 1, 0, 3)
            )
            wp_my = wp[e][h0 : h0 + HS, :]                  # [HS, D]
            # wpr[p, d, k, c2] = wp_my[k*128+p, d*128+c2]
            wpr_e.append(
                wp_my.reshape(MG, 128, KD, 128).transpose(1, 2, 0, 3)
            )
        wvr = np.ascontiguousarray(np.concatenate(wvr_e, axis=0).astype(BF16NP))
        wpr = np.ascontiguousarray(np.stack(wpr_e, axis=0).astype(BF16NP))
        in_maps.append({"xtr": xtr, "wvr": wvr, "wpr": wpr})
    return in_maps


def combine(results, order):
    """Sum the 8 hidden-slice partial outputs and inverse-permute."""
    ysum = np.zeros((128, KD, T), dtype=np.float32)
    for r in results:
        ysum += np.asarray(r["yt"]).astype(np.float32)
    yT = ysum.transpose(1, 0, 2).reshape(D, T)  # [D, T] compact order
    out = np.empty((T, D), dtype=np.float32)
    out[order] = yT.T
    return out.reshape(2, 2048, D)


def kernel(x, w_router, w_v, w_proj):
    eidx, counts, order = _route(x, w_router)
    nc = _get_nc(counts)
    in_maps = make_in_maps(x, w_v, w_proj, order)
    res = run_bass_kernel_spmd(nc, in_maps, core_ids=list(range(8)), trace=False)
    return combine(res.results, order)


if __name__ == "__main__":
    sys.path.insert(0, "/root/problem")
    import reference

    ins = {k: np.asarray(v) for k, v in reference.setup_inputs().items()}
    got = kernel(**ins)
    exp = np.asarray(reference.reference(**ins))
    err = np.abs(got - exp)
    denom = np.abs(exp).max()
    print("max abs err:", err.max(), "rel:", err.max() / denom)


# revision 7
# speedup vs baseline: 2.3450x; 1.0081x over previous
"""MoE top-1 routing kernel for Trainium2, 8 NeuronCores.

Problem: x [2, 2048, 1024] f32; router w [1024, 4]; per-expert SwiGLU MLP
  gv = x @ w_v[e] ([1024, 8192]); h = silu(gv[:, :4096]) * gv[:, 4096:];
  y = h @ w_proj[e] ([4096, 1024]); out[t] = y_{argmax(router)}[t].

Sharding: tokens are dispatched by expert_idx at the host sharding step
(router is 0.03% of total FLOPs; argmax computed in f64, which matches the
f32 reference argmax exactly -- min top-2 logit gap for this data is ~3e-4,
far above f32 rounding noise). Tokens are permuted into expert-contiguous
order; every core receives ALL tokens (transposed, bf16) plus a 1/8 slice
of the hidden dimension of EVERY expert's weights (hidden-slice model
parallelism). Per-core work is therefore exactly total_tokens * (3*D*H/8)
MACs regardless of expert load imbalance, with zero capacity padding:
matmul free dims are the ragged per-expert block lengths.

Device program (identical SPMD; per-core weight inputs):
  for e in experts:  # token blocks of <=512 columns of the compact stream
    for hm in 4 gate/value 128-row tile pairs:
      psg = sum_k wv_gate[k] @ xT[k, blk]; psv = sum_k wv_val[k] @ xT[k, blk]
      ht[hm, blk] = silu(psg) * psv          (bf16)
    for blk: for d in 8: psy[d] = sum_k wp[d,k] @ ht[k, blk] -> yt (bf16)
Host combines: out = sum over cores of yt (f32), inverse-permuted.

All matmuls are bf16 (1 cycle/row at any free size on TRN2), PSUM f32.
PE work per core = 4096 tokens * 96 cycles = ~394k cycles = ~164 us.
"""

import sys

sys.path.insert(0, "/opt/trn_rl_repo")

import ml_dtypes
import numpy as np

import concourse.bass as bass  # noqa: F401  (kept for parity with utils)
import concourse.mybir as mybir
import concourse.tile as tile
from concourse import bacc
from concourse.bass_utils import run_bass_kernel_spmd

F32 = mybir.dt.float32
BF16 = mybir.dt.bfloat16
AF = mybir.ActivationFunctionType
OP = mybir.AluOpType
BF16NP = np.dtype(ml_dtypes.bfloat16)

T = 4096      # tokens
D = 1024      # model dim
E = 4         # experts
H = 4096      # MLP hidden (SwiGLU: w_v outputs 2*H)
HS = H // 8   # hidden slice per core
KD = D // 128     # 8 k-tiles over model dim
MG = HS // 128    # 4 gate (and 4 value) 128-row tiles per slice
BLK = 512         # max token block (one PSUM bank of f32)

# Expert loads for the seed-0 reference data (default build).
DEFAULT_COUNTS = (1149, 902, 974, 1071)


def _blocks(counts):
    """Static block structure: (expert, col_start, col_len) over the compact
    token stream; ragged tails, no padding. Expert 0 leads with a small
    128-col block so the PE can start ~3us earlier (first DMA is smaller)."""
    out = []
    c0 = 0
    for e in range(E):
        n = int(counts[e])
        off = 0
        if e == 0 and n > 256:
            out.append((e, c0, 256))
            off = 256
        while off < n:
            ln = min(BLK, n - off)
            out.append((e, c0 + off, ln))
            off += ln
        c0 += n
    return out


def _build(counts):
    nc = bacc.Bacc("TRN2", target_bir_lowering=False, debug=False, num_devices=8)

    xtr_d = nc.dram_tensor("xtr", [128, KD, T], BF16, kind="ExternalInput").ap()
    wvr_d = nc.dram_tensor(
        "wvr", [E * 2 * MG, 128, KD, 128], BF16, kind="ExternalInput"
    ).ap()
    wpr_d = nc.dram_tensor(
        "wpr", [E, 128, KD, MG, 128], BF16, kind="ExternalInput"
    ).ap()
    yt_d = nc.dram_tensor("yt", [128, KD, T], BF16, kind="ExternalOutput").ap()

    blocks = _blocks(counts)

    with tile.TileContext(nc) as tc:
        with (
            tc.tile_pool(name="xte", bufs=1) as xp,
            tc.tile_pool(name="ht", bufs=1) as hp,
            tc.tile_pool(name="wv", bufs=6) as wvp,
            tc.tile_pool(name="wp", bufs=2) as wpp,
            tc.tile_pool(name="act", bufs=3) as actp,
            tc.tile_pool(name="out", bufs=3) as outp,
            tc.tile_pool(name="pg", bufs=2, space="PSUM") as pg,
            tc.tile_pool(name="pv", bufs=3, space="PSUM") as pv,
            tc.tile_pool(name="py", bufs=3, space="PSUM") as py,
        ):
            xte = xp.tile([128, KD, T], BF16)
            ht = hp.tile([128, MG, T], BF16)

            # Input + weight DMAs all on the SP queue, issued in consumption
            # order so the shared DMA engines serve first-needed first.
            # Per-expert: this expert's x columns, then its gate/value tile
            # pairs, then its proj tile.
            wv_tiles = {}
            wp_tiles = {}

            def load_wv(e, hm):
                wg = wvp.tile([128, KD, 128], BF16, tag="wg")
                nc.sync.dma_start(wg[:], wvr_d[e * 2 * MG + hm])
                wl = wvp.tile([128, KD, 128], BF16, tag="wl")
                nc.sync.dma_start(wl[:], wvr_d[e * 2 * MG + MG + hm])
                wv_tiles[(e, hm)] = (wg, wl)

            # First weight pair before any x so Ldweights overlaps the x DMA.
            load_wv(0, 0)
            for e in range(E):
                for (ee, c0, ln) in blocks:
                    if ee != e:
                        continue
                    nc.sync.dma_start(
                        xte[:, :, c0 : c0 + ln], xtr_d[:, :, c0 : c0 + ln]
                    )
                for hm in range(MG):
                    if (e, hm) in wv_tiles:
                        continue
                    load_wv(e, hm)
                wp_sb = wpp.tile([128, KD, MG, 128], BF16, tag="wp")
                nc.sync.dma_start(wp_sb[:], wpr_d[e])
                wp_tiles[e] = wp_sb

            for e in range(E):
                eblocks = [b for b in blocks if b[0] == e]
                # gate/value matmuls + silu-mult into ht
                for hm in range(MG):
                    wg, wl = wv_tiles[(e, hm)]
                    for (_, c0, ln) in eblocks:
                        psg = pg.tile([128, BLK], F32, tag="g")
                        for k in range(KD):
                            nc.tensor.matmul(
                                psg[:, :ln],
                                lhsT=wg[:, k, :],
                                rhs=xte[:, k, c0 : c0 + ln],
                                start=(k == 0),
                                stop=(k == KD - 1),
                            )
                        psv = pv.tile([128, BLK], F32, tag="v")
                        for k in range(KD):
                            nc.tensor.matmul(
                                psv[:, :ln],
                                lhsT=wl[:, k, :],
                                rhs=xte[:, k, c0 : c0 + ln],
                                start=(k == 0),
                                stop=(k == KD - 1),
                            )
                        sact = actp.tile([128, BLK], F32, tag="s")
                        nc.scalar.activation(sact[:, :ln], psg[:, :ln], AF.Silu)
                        nc.vector.tensor_tensor(
                            out=ht[:, hm, c0 : c0 + ln],
                            in0=sact[:, :ln],
                            in1=psv[:, :ln],
                            op=OP.mult,
                        )
                # proj: per token block, all 8 d-tiles, one output DMA
                wp_sb = wp_tiles[e]
                for (_, c0, ln) in eblocks:
                    ysb = outp.tile([128, KD, BLK], BF16, tag="y")
                    for d in range(KD):
                        psy = py.tile([128, BLK], F32, tag="py")
                        for k in range(MG):
                            nc.tensor.matmul(
                                psy[:, :ln],
                                lhsT=wp_sb[:, d, k, :],
                                rhs=ht[:, k, c0 : c0 + ln],
                                start=(k == 0),
                                stop=(k == MG - 1),
                            )
                        nc.vector.tensor_copy(ysb[:, d, :ln], psy[:, :ln])
                    is_last = (e, c0, ln) == blocks[-1]
                    # last block: HWDGE on the (idle) Act queue beats SWDGE's
                    # ~1.3us fixed overhead on the critical tail
                    dma_eng = nc.scalar if is_last else nc.gpsimd
                    dma_eng.dma_start(
                        yt_d[:, :, c0 : c0 + ln], ysb[:, :, :ln]
                    )

    nc.compile()
    return nc


_NC = None
_NC_COUNTS = None


def _route(x, w_router):
    """Host router: f64 logits argmax (exactly matches the f32 reference
    argmax for any non-degenerate top-2 gap)."""
    x2 = np.asarray(x, dtype=np.float64).reshape(T, D)
    logits = x2 @ np.asarray(w_router, dtype=np.float64)
    eidx = np.argmax(logits, axis=1)
    counts = np.bincount(eidx, minlength=E)
    order = np.argsort(eidx, kind="stable")
    return eidx, counts, order


def _get_nc(counts=DEFAULT_COUNTS):
    global _NC, _NC_COUNTS
    counts = tuple(int(c) for c in counts)
    if _NC is None or _NC_COUNTS != counts:
        _NC = _build(counts)
        _NC_COUNTS = counts
    return _NC


def make_in_maps(x, w_v, w_proj, order):
    x2 = np.asarray(x, dtype=np.float32).reshape(T, D)
    wv = np.asarray(w_v, dtype=np.float32)
    wp = np.asarray(w_proj, dtype=np.float32)

    # compact transposed x, bf16: xtr[p, k, t] = x[order[t], k*128+p]
    xT = np.ascontiguousarray(x2[order].T)  # [D, T]
    xtr = np.ascontiguousarray(
        xT.reshape(KD, 128, T).transpose(1, 0, 2).astype(BF16NP)
    )

    in_maps = []
    for c in range(8):
        h0 = c * HS
        wvr_e = []
        wpr_e = []
        for e in range(E):
            gate = wv[e][:, h0 : h0 + HS]                   # [D, HS]
            val = wv[e][:, H + h0 : H + h0 + HS]            # [D, HS]
            wv_my = np.concatenate([gate, val], axis=1)     # [D, 2*HS]
            # wvr[m, p, k, c2] = wv_my[k*128+p, m*128+c2]
            wvr_e.append(
                wv_my.reshape(KD, 128, 2 * MG, 128).transpose(2, 1, 0, 3)
            )
            wp_my = wp[e][h0 : h0 + HS, :]                  # [HS, D]
            # wpr[p, d, k, c2] = wp_my[k*128+p, d*128+c2]
            wpr_e.append(
                wp_my.reshape(MG, 128, KD, 128).transpose(1, 2, 0, 3)
            )
        wvr = np.ascontiguousarray(np.concatenate(wvr_e, axis=0).astype(BF16NP))
        wpr = np.ascontiguousarray(np.stack(wpr_e, axis=0).astype(BF16NP))
        in_maps.append({"xtr": xtr, "wvr": wvr, "wpr": wpr})
    return in_maps


def combine(results, order):
    """Sum the 8 hidden-slice partial outputs and inverse-permute."""
    ysum = np.zeros((128, KD, T), dtype=np.float32)
    for r in results:
        ysum += np.asarray(r["yt"]).astype(np.float32)
    yT = ysum.transpose(1, 0, 2).reshape(D, T)  # [D, T] compact order
    out = np.empty((T, D), dtype=np.float32)
    out[order] = yT.T
    return out.reshape(2, 2048, D)


def kernel(x, w_router, w_v, w_proj):
    eidx, counts, order = _route(x, w_router)
    nc = _get_nc(counts)
    in_maps = make_in_maps(x, w_v, w_proj, order)
    res = run_bass_kernel_spmd(nc, in_maps, core_ids=list(range(8)), trace=False)
    return combine(res.results, order)


if __name__ == "__main__":
    sys.path.insert(0, "/root/problem")
    import reference

    ins = {k: np.asarray(v) for k, v in reference.setup_inputs().items()}
    got = kernel(**ins)
    exp = np.asarray(reference.reference(**ins))
    err = np.abs(got - exp)
    denom = np.abs(exp).max()
    print("max abs err:", err.max(), "rel:", err.max() / denom)


# revision 22
# speedup vs baseline: 2.3976x; 1.0224x over previous
"""MoE top-1 routing kernel for Trainium2, 8 NeuronCores.

Problem: x [2, 2048, 1024] f32; router w [1024, 4]; per-expert SwiGLU MLP
  gv = x @ w_v[e] ([1024, 8192]); h = silu(gv[:, :4096]) * gv[:, 4096:];
  y = h @ w_proj[e] ([4096, 1024]); out[t] = y_{argmax(router)}[t].

Sharding: tokens are dispatched by expert_idx at the host sharding step
(router is 0.03% of total FLOPs; argmax computed in f64, which matches the
f32 reference argmax exactly -- min top-2 logit gap for this data is ~3e-4,
far above f32 rounding noise). Tokens are permuted into expert-contiguous
order; every core receives ALL tokens (transposed, bf16) plus a 1/8 slice
of the hidden dimension of EVERY expert's weights (hidden-slice model
parallelism). Per-core work is therefore exactly total_tokens * (3*D*H/8)
MACs regardless of expert load imbalance, with zero capacity padding:
matmul free dims are the ragged per-expert block lengths.

Device program (identical SPMD; per-core weight inputs):
  for e in experts:  # token blocks of <=512 columns of the compact stream
    for hm in 4 gate/value 128-row tile pairs:
      psg = sum_k wv_gate[k] @ xT[k, blk]; psv = sum_k wv_val[k] @ xT[k, blk]
      ht[hm, blk] = silu(psg) * psv          (bf16)
    for blk: for d in 8: psy[d] = sum_k wp[d,k] @ ht[k, blk] -> yt (bf16)
Host combines: out = sum over cores of yt (f32), inverse-permuted.

All matmuls are bf16 (1 cycle/row at any free size on TRN2), PSUM f32.
PE work per core = 4096 tokens * 96 cycles = ~394k cycles = ~164 us.
"""

import sys

sys.path.insert(0, "/opt/trn_rl_repo")

import ml_dtypes
import numpy as np

import concourse.bass as bass  # noqa: F401  (kept for parity with utils)
import concourse.mybir as mybir
import concourse.tile as tile
from concourse import bacc
from concourse.bass_utils import run_bass_kernel_spmd

F32 = mybir.dt.float32
BF16 = mybir.dt.bfloat16
AF = mybir.ActivationFunctionType
OP = mybir.AluOpType
BF16NP = np.dtype(ml_dtypes.bfloat16)

T = 4096      # tokens
D = 1024      # model dim
E = 4         # experts
H = 4096      # MLP hidden (SwiGLU: w_v outputs 2*H)
HS = H // 8   # hidden slice per core
KD = D // 128     # 8 k-tiles over model dim
MG = HS // 128    # 4 gate (and 4 value) 128-row tiles per slice
BLK = 512         # max token block (one PSUM bank of f32)
NWARM = 26        # PE warm-up dummy matmuls (cover the startup DMA window)

# Expert loads for the seed-0 reference data (default build).
DEFAULT_COUNTS = (1149, 902, 974, 1071)


def _blocks(counts):
    """Static block structure: (expert, col_start, col_len) over the compact
    token stream; ragged tails, no padding. Expert 0 leads with a small
    128-col block so the PE can start ~3us earlier (first DMA is smaller)."""
    out = []
    c0 = 0
    for e in range(E):
        n = int(counts[e])
        off = 0
        if e == 0 and n > 256:
            out.append((e, c0, 256))
            off = 256
        while off < n:
            ln = min(BLK, n - off)
            out.append((e, c0 + off, ln))
            off += ln
        c0 += n
    return out


def _build(counts):
    nc = bacc.Bacc("TRN2", target_bir_lowering=False, debug=False, num_devices=8)

    xtr_d = nc.dram_tensor("xtr", [128, KD, T], BF16, kind="ExternalInput").ap()
    wvr_d = nc.dram_tensor(
        "wvr", [E * 2 * MG, 128, KD, 128], BF16, kind="ExternalInput"
    ).ap()
    wpr_d = nc.dram_tensor(
        "wpr", [E, 128, KD, MG, 128], BF16, kind="ExternalInput"
    ).ap()
    yt_d = nc.dram_tensor("yt", [128, KD, T], BF16, kind="ExternalOutput").ap()

    blocks = _blocks(counts)

    with tile.TileContext(nc) as tc:
        with (
            tc.tile_pool(name="xte", bufs=1) as xp,
            tc.tile_pool(name="ht", bufs=1) as hp,
            tc.tile_pool(name="wv", bufs=6) as wvp,
            tc.tile_pool(name="wp", bufs=2) as wpp,
            tc.tile_pool(name="act", bufs=3) as actp,
            tc.tile_pool(name="out", bufs=3) as outp,
            tc.tile_pool(name="pg", bufs=2, space="PSUM") as pg,
            tc.tile_pool(name="pv", bufs=2, space="PSUM") as pv,
            tc.tile_pool(name="py", bufs=4, space="PSUM") as py,
        ):
            xte = xp.tile([128, KD, T], BF16)
            ht = hp.tile([128, MG, T], BF16)

            # PE warm-up: the Tensor engine runs at half clock until it has
            # been continuously busy for 3us. Dummy matmuls on a memset tile
            # keep it busy through the startup DMA window so all real
            # matmuls run at full p-state.
            warm = actp.tile([128, 128], BF16, tag="warm")
            nc.vector.memset(warm[:], 0.0)
            pwu = pg.tile([128, 128], F32, tag="g")
            for _ in range(NWARM):
                nc.tensor.matmul(
                    pwu[:], lhsT=warm[:], rhs=warm[:], start=True, stop=True
                )

            # Startup-critical DMAs on three different queues (per-DMA
            # sequencer time is ~0.6us, so serializing them on one queue
            # delays the first matmul); everything else in consumption order
            # on the SP queue.
            wv_tiles = {}
            wp_tiles = {}

            def load_wv(e, hm, eng_g=None, eng_l=None):
                wg = wvp.tile([128, KD, 128], BF16, tag="wg")
                (eng_g or nc.sync).dma_start(wg[:], wvr_d[e * 2 * MG + hm])
                wl = wvp.tile([128, KD, 128], BF16, tag="wl")
                (eng_l or nc.sync).dma_start(wl[:], wvr_d[e * 2 * MG + MG + hm])
                wv_tiles[(e, hm)] = (wg, wl)

            first_blk = blocks[0]
            # wl on the Pool SWDGE queue: its slower issue path lands it on
            # the shared DMA engines after the 2nd x chunk, which matches
            # consumption order (psv needs it ~0.9us after psg starts).
            load_wv(0, 0, eng_g=nc.sync, eng_l=nc.gpsimd)
            _, fc0, fln = first_blk
            nc.scalar.dma_start(
                xte[:, :, fc0 : fc0 + fln], xtr_d[:, :, fc0 : fc0 + fln]
            )
            for e in range(E):
                for (ee, c0, ln) in blocks:
                    if ee != e:
                        continue
                    if (ee, c0, ln) == first_blk:
                        continue
                    nc.sync.dma_start(
                        xte[:, :, c0 : c0 + ln], xtr_d[:, :, c0 : c0 + ln]
                    )
                for hm in range(MG):
                    if (e, hm) in wv_tiles:
                        continue
                    load_wv(e, hm)
                wp_sb = wpp.tile([128, KD, MG, 128], BF16, tag="wp")
                nc.sync.dma_start(wp_sb[:], wpr_d[e])
                wp_tiles[e] = wp_sb

            for e in range(E):
                eblocks = [b for b in blocks if b[0] == e]
                # gate/value matmuls + silu-mult into ht
                for hm in range(MG):
                    wg, wl = wv_tiles[(e, hm)]
                    for (_, c0, ln) in eblocks:
                        psg = pg.tile([128, BLK], F32, tag="g")
                        for k in range(KD):
                            nc.tensor.matmul(
                                psg[:, :ln],
                                lhsT=wg[:, k, :],
                                rhs=xte[:, k, c0 : c0 + ln],
                                start=(k == 0),
                                stop=(k == KD - 1),
                            )
                        psv = pv.tile([128, BLK], F32, tag="v")
                        for k in range(KD):
                            nc.tensor.matmul(
                                psv[:, :ln],
                                lhsT=wl[:, k, :],
                                rhs=xte[:, k, c0 : c0 + ln],
                                start=(k == 0),
                                stop=(k == KD - 1),
                            )
                        sact = actp.tile([128, BLK], F32, tag="s")
                        nc.scalar.activation(sact[:, :ln], psg[:, :ln], AF.Silu)
                        nc.vector.tensor_tensor(
                            out=ht[:, hm, c0 : c0 + ln],
                            in0=sact[:, :ln],
                            in1=psv[:, :ln],
                            op=OP.mult,
                        )
                # proj: per token block, all 8 d-tiles, one output DMA
                wp_sb = wp_tiles[e]
                for (_, c0, ln) in eblocks:
                    ysb = outp.tile([128, KD, BLK], BF16, tag="y")
                    is_last = (e, c0, ln) == blocks[-1]
                    for d in range(KD):
                        psy = py.tile([128, BLK], F32, tag="py")
                        for k in range(MG):
                            nc.tensor.matmul(
                                psy[:, :ln],
                                lhsT=wp_sb[:, d, k, :],
                                rhs=ht[:, k, c0 : c0 + ln],
                                start=(k == 0),
                                stop=(k == MG - 1),
                            )
                        if is_last and d % 2 == 1 and d != KD - 1:
                            # final block: alternate copies onto the idle Act
                            # engine so the exit chain isn't DVE-serialized
                            nc.scalar.activation(
                                ysb[:, d, :ln], psy[:, :ln], AF.Copy
                            )
                        else:
                            nc.vector.tensor_copy(ysb[:, d, :ln], psy[:, :ln])
                        if is_last and d == KD - 2:
                            # drain d0..6 early so only d7's copy + a tiny
                            # DMA sit on the critical tail
                            nc.scalar.dma_start(
                                yt_d[:, : KD - 1, c0 : c0 + ln],
                                ysb[:, : KD - 1, :ln],
                            )
                    if is_last:
                        nc.sync.dma_start(
                            yt_d[:, KD - 1 :, c0 : c0 + ln],
                            ysb[:, KD - 1 :, :ln],
                        )
                    else:
                        nc.scalar.dma_start(
                            yt_d[:, :, c0 : c0 + ln], ysb[:, :, :ln]
                        )

    nc.compile()
    return nc


_NC = None
_NC_COUNTS = None


def _route(x, w_router):
    """Host router: f64 logits argmax (exactly matches the f32 reference
    argmax for any non-degenerate top-2 gap)."""
    x2 = np.asarray(x, dtype=np.float64).reshape(T, D)
    logits = x2 @ np.asarray(w_router, dtype=np.float64)
    eidx = np.argmax(logits, axis=1)
    counts = np.bincount(eidx, minlength=E)
    order = np.argsort(eidx, kind="stable")
    return eidx, counts, order


def _get_nc(counts=DEFAULT_COUNTS):
    global _NC, _NC_COUNTS
    counts = tuple(int(c) for c in counts)
    if _NC is None or _NC_COUNTS != counts:
        _NC = _build(counts)
        _NC_COUNTS = counts
    return _NC


def make_in_maps(x, w_v, w_proj, order):
    x2 = np.asarray(x, dtype=np.float32).reshape(T, D)
    wv = np.asarray(w_v, dtype=np.float32)
    wp = np.asarray(w_proj, dtype=np.float32)

    # compact transposed x, bf16: xtr[p, k, t] = x[order[t], k*128+p]
    xT = np.ascontiguousarray(x2[order].T)  # [D, T]
    xtr = np.ascontiguousarray(
        xT.reshape(KD, 128, T).transpose(1, 0, 2).astype(BF16NP)
    )

    in_maps = []
    for c in range(8):
        h0 = c * HS
        wvr_e = []
        wpr_e = []
        for e in range(E):
            gate = wv[e][:, h0 : h0 + HS]                   # [D, HS]
            val = wv[e][:, H + h0 : H + h0 + HS]            # [D, HS]
            wv_my = np.concatenate([gate, val], axis=1)     # [D, 2*HS]
            # wvr[m, p, k, c2] = wv_my[k*128+p, m*128+c2]
            wvr_e.append(
                wv_my.reshape(KD, 128, 2 * MG, 128).transpose(2, 1, 0, 3)
            )
            wp_my = wp[e][h0 : h0 + HS, :]                  # [HS, D]
            # wpr[p, d, k, c2] = wp_my[k*128+p, d*128+c2]
            wpr_e.append(
                wp_my.reshape(MG, 128, KD, 128).transpose(1, 2, 0, 3)
            )
        wvr = np.ascontiguousarray(np.concatenate(wvr_e, axis=0).astype(BF16NP))
        wpr = np.ascontiguousarray(np.stack(wpr_e, axis=0).astype(BF16NP))
        in_maps.append({"xtr": xtr, "wvr": wvr, "wpr": wpr})
    return in_maps


def combine(results, order):
    """Sum the 8 hidden-slice partial outputs and inverse-permute."""
    ysum = np.zeros((128, KD, T), dtype=np.float32)
    for r in results:
        ysum += np.asarray(r["yt"]).astype(np.float32)
    yT = ysum.transpose(1, 0, 2).reshape(D, T)  # [D, T] compact order
    out = np.empty((T, D), dtype=np.float32)
    out[order] = yT.T
    return out.reshape(2, 2048, D)


def kernel(x, w_router, w_v, w_proj):
    eidx, counts, order = _route(x, w_router)
    nc = _get_nc(counts)
    in_maps = make_in_maps(x, w_v, w_proj, order)
    res = run_bass_kernel_spmd(nc, in_maps, core_ids=list(range(8)), trace=False)
    return combine(res.results, order)


if __name__ == "__main__":
    sys.path.insert(0, "/root/problem")
    import reference

    ins = {k: np.asarray(v) for k, v in reference.setup_inputs().items()}
    got = kernel(**ins)
    exp = np.asarray(reference.reference(**ins))
    err = np.abs(got - exp)
    denom = np.abs(exp).max()
    print("max abs err:", err.max(), "rel:", err.max() / denom)
